# revision 2
# baseline (speedup 1.0000x reference)
"""Maxwell viscoelastic recurrence (explicit Euler) on 8 TRN2 NeuronCores.

Math: with E_inf=0.5, E=2.0, eta=1.0,
    d_n        = eps_n - gamma_n
    sig_n      = 0.5*eps_n + 2*d_n              = 2.5*eps_n - 2*gamma_n
    gamma_{n+1}= gamma_n + 2*dt_n*d_n           = (1-2*dt_n)*gamma_n + 2*dt_n*eps_n

Substituting h_n = -2*gamma_n gives a first-order linear scan:
    h_{n+1} = a_n*h_n + b_n,  a_n = 1-2*dt_n,  b_n = -4*dt_n*eps_n,  h_0 = 0
    sig_n   = 2.5*eps_n + h_n

which maps directly onto the VectorEngine's tensor_tensor_scan
(state = data0*state + data1 along the free axis, one recurrence per
partition lane).

Sharding: pure data parallel over the batch axis (2048 rows -> 256 per
core). Per core the 256 rows form two 128-partition tiles; T=8192 is
streamed in column chunks with the scan carry chained via the chunk's
leading h column.
"""

import numpy as np

B, T = 2048, 8192
N_CORES = 8
B_LOCAL = B // N_CORES  # 256
P = 128                 # SBUF partitions
C = 2048                # T-chunk columns
N_PT = B_LOCAL // P     # partition tiles per core
N_CH = T // C           # chunks along T

_cache = {}


def _build():
    import concourse.tile as tile
    from concourse import bacc, mybir

    f32 = mybir.dt.float32
    mult = mybir.AluOpType.mult
    add = mybir.AluOpType.add

    nc = bacc.Bacc("TRN2", target_bir_lowering=False, debug=False,
                   num_devices=N_CORES)
    eps_d = nc.dram_tensor("eps", [B_LOCAL, T], f32, kind="ExternalInput").ap()
    dts_d = nc.dram_tensor("dts", [B_LOCAL, T], f32, kind="ExternalInput").ap()
    out_d = nc.dram_tensor("out", [B_LOCAL, T], f32, kind="ExternalOutput").ap()

    with tile.TileContext(nc) as tc:
        with (
            tc.tile_pool(name="io", bufs=3) as io_pool,
            tc.tile_pool(name="ab", bufs=3) as ab_pool,
            tc.tile_pool(name="h", bufs=2 * N_PT) as h_pool,
            tc.tile_pool(name="misc", bufs=1) as misc_pool,
        ):
            one = misc_pool.tile([P, 1], f32, tag="one")
            nc.gpsimd.memset(one[:], 1.0)

            h_prev = [None] * N_PT
            for c in range(N_CH):
                for pt in range(N_PT):
                    r0 = pt * P
                    rows = slice(r0, r0 + P)
                    cols = slice(c * C, (c + 1) * C)

                    eps_t = io_pool.tile([P, C], f32, tag="eps")
                    nc.sync.dma_start(eps_t[:], eps_d[rows, cols])
                    dts_t = io_pool.tile([P, C], f32, tag="dts")
                    nc.sync.dma_start(dts_t[:], dts_d[rows, cols])

                    # a = 1 - 2*dt   (ScalarE: Identity(dt*-2 + 1))
                    a_t = ab_pool.tile([P, C], f32, tag="a")
                    nc.scalar.activation(
                        a_t[:], dts_t[:],
                        mybir.ActivationFunctionType.Identity,
                        bias=one[:], scale=-2.0,
                    )
                    # b = -4*dt*eps  (VectorE: (dt*-4)*eps)
                    b_t = ab_pool.tile([P, C], f32, tag="b")
                    nc.vector.scalar_tensor_tensor(
                        b_t[:], dts_t[:], -4.0, eps_t[:], mult, mult)

                    # h chunk: col 0 carries h at chunk start, scan fills 1..C
                    h_t = h_pool.tile([P, C + 1], f32, tag="h")
                    if c == 0:
                        nc.gpsimd.memset(h_t[:, 0:1], 0.0)
                    else:
                        nc.scalar.copy(h_t[:, 0:1], h_prev[pt][:, C:C + 1])
                    nc.vector.tensor_tensor_scan(
                        h_t[:, 1:C + 1], a_t[:], b_t[:], h_t[:, 0:1],
                        mult, add)
                    h_prev[pt] = h_t

                    # sig = 2.5*eps + h   (VectorE)
                    sig_t = io_pool.tile([P, C], f32, tag="sig")
                    nc.vector.scalar_tensor_tensor(
                        sig_t[:], eps_t[:], 2.5, h_t[:, 0:C], mult, add)
                    nc.sync.dma_start(out_d[rows, cols], sig_t[:])

    nc.compile()
    return nc


def kernel(eps: np.ndarray, dts: np.ndarray) -> np.ndarray:
    from concourse.bass_utils import run_bass_kernel_spmd

    e = np.ascontiguousarray(eps.reshape(B, T), dtype=np.float32)
    d = np.ascontiguousarray(dts.reshape(B, T), dtype=np.float32)

    if "nc" not in _cache:
        _cache["nc"] = _build()
    nc = _cache["nc"]

    in_maps = [
        {"eps": e[i * B_LOCAL:(i + 1) * B_LOCAL],
         "dts": d[i * B_LOCAL:(i + 1) * B_LOCAL]}
        for i in range(N_CORES)
    ]
    res = run_bass_kernel_spmd(nc, in_maps, core_ids=list(range(N_CORES)))
    out = np.concatenate(
        [np.asarray(res.results[i]["out"]) for i in range(N_CORES)], axis=0)
    return out.reshape(B, T, 1)
